# revision 33
# baseline (speedup 1.0000x reference)
"""BLEU-precision loss kernel for Trainium2 (8 NeuronCores, data parallel).

loss = 1 - mean_i |set(pred_i) & set(tgt_i)| / |set(pred_i)|   over 64 rows.

Per core (8 rows, 16 row-sides), a presence grid over the 32000-id vocab is
built for every row-side by one GPSIMD `dma_scatter_add` (SBUF-destination
parity mode, tokens_per_rank=128): idx = raw token id (int16), payload =
constant 1.0 bf16. Each token lands in cell (partition = t&127, col = t>>8)
of one of two [128,125] half-grids selected by bit 7 of t, so a cell is
nonzero iff the id occurs in the row-side. Indices come from int32->int16
tensor_copy casts (exact for ids < 2^15); only GPSIMD group 0 of the
[128, N] index tile carries real data, the rest is zero-filled once.

Every engine pulls its weight concurrently:
  SP:   three 2-row token loads, final result store.
  ACT:  three 2-row token loads, then |set(pred)| for rows 5..7 via
        Sign+accumulate (a dummy activation absorbs the act-table load
        during its idle window).
  DVE:  payload ones, pred-grid zeros, all eight index casts, |set(pred)|
        for rows 0..4 (is_gt+accum), overlap rows 0..3 (logical_and then
        mult+accum).
  Pool: two 2-row loads, index-tile fill, tgt-grid zeros, the 16 scatters,
        overlap rows 4..7.

Host sums the [128,16] f32 per-partition partials and finishes the exact
precision / mean arithmetic.
"""
import sys

sys.path.insert(0, "/opt/trn_rl_repo")

import numpy as np
import concourse.bass as bass
import concourse.bacc as bacc
from concourse import mybir

B = 64          # total rows
ROWS = 8        # rows per core
N_CORES = 8
L = 2048        # tokens per row
V = 32000       # vocab
P = 128
F32 = mybir.dt.float32
BF16 = mybir.dt.bfloat16
I32 = mybir.dt.int32
I16 = mybir.dt.int16

# scatter order: all pred rows, then tgt rows 4..7 (whose casts complete
# early via the Pool-loaded halves), then tgt rows 0..3
SC_SIDES = [(r, 0) for r in range(ROWS)] + \
    [(4, 1), (5, 1), (0, 1), (1, 1), (2, 1), (3, 1), (6, 1), (7, 1)]
SC_J = {rs: j for j, rs in enumerate(SC_SIDES)}
NSC = len(SC_SIDES)  # 16

DVE_SS_ROWS = (0, 1, 2, 3, 4, 5, 6, 7)
# overlap rows ordered to match tgt-scatter completion order
DVE_OV_ROWS = (4, 5, 0, 1, 2, 3, 6, 7)

_CACHE = {}


def _grid_blk(row, side):
    return 2 * row + side


def _build_kernel():
    nc = bacc.Bacc()
    pred = nc.dram_tensor("pred", [ROWS, L], I32, kind="ExternalInput")
    tgt = nc.dram_tensor("tgt", [ROWS, L], I32, kind="ExternalInput")
    out = nc.dram_tensor("out", [P, 16], F32, kind="ExternalOutput")

    from contextlib import ExitStack

    es = ExitStack()
    with es:
        sb = lambda name, shape, dt: es.enter_context(nc.sbuf_tensor(name, shape, dt))
        sem = lambda name: es.enter_context(nc.semaphore(name))

        tok32 = sb("tok32", [16, 2048], I32)     # pred cols 0:1024, tgt 1024:2048
        idx16 = sb("idx16", [P, NSC * 128], I16)
        ones = sb("ones", [P, 16], BF16)
        grid = sb("grid", [P, NSC * 256], BF16)
        junkV = sb("junkV", [P, ROWS * 256], BF16)
        junkO = sb("junkO", [P, ROWS * 256], BF16)
        junkS = sb("junkS", [P, ROWS * 256], BF16)
        res = sb("res", [P, 16], F32)

        # per-2-row load/cast sems: pred halves a..d, tgt halves a..d
        s_ld = {k: sem(f"s_ld_{k}") for k in
                ("pa", "pb", "pc", "pd", "ta", "tb", "tc", "td")}
        s_cast = {k: sem(f"s_cast_{k}") for k in
                  ("pa", "pb", "pc", "pd", "ta", "tb", "tc", "td")}
        s_jf = sem("s_jf")
        s_zp = sem("s_zp")       # DVE: ones + pred-grid zeros
        s_zt = sem("s_zt")       # Pool: tgt-grid zeros
        s_dve = sem("s_dve")
        s_ss = sem("s_ss")       # ACT setsize Signs
        s_povl = sem("s_povl")   # Pool overlap tail ops
        s_sc = [sem(f"s_sc{j}") for j in range(NSC)]
        s_out = sem("s_out")

        block = es.enter_context(nc.Block())

        ticks = {"final": 0}

        def seg2(t, blk):
            a = t[:]
            return bass.AP(a.tensor, a.offset + blk * 256,
                           [a.ap[0], [128, 2], [1, 125]])

        def zeros_ap(side):
            a = grid[:].bitcast(I32)
            return bass.AP(a.tensor, a.offset + side * 128,
                           [a.ap[0], [256, ROWS], [1, 128]])

        def ld2(eng, side_t, coloff, rows0, s):
            # load rows0..rows0+2 of one side into tok32
            eng.dma_start(
                out=bass.AP(tok32[:].tensor, coloff + rows0 * 128,
                            [[2048, 16], [128, 2], [1, 128]]),
                in_=bass.AP(side_t[:].tensor, rows0 * L,
                            [[128, 16], [L, 2], [1, 128]]),
            ).then_inc(s, 16)

        # (key, side, rows0): cast granules; tok col = side*1024 + rows0*128
        GRAN = [("pa", 0, 0), ("pb", 0, 2), ("pc", 0, 4), ("pd", 0, 6),
                ("ta", 1, 0), ("tb", 1, 2), ("tc", 1, 4), ("td", 1, 6)]
        GKEY = {(s, r0): k for k, s, r0 in GRAN}

        def cast_key(row, side):
            return GKEY[(side, (row // 2) * 2)]

        # ---------------- DVE ----------------
        @block.vector
        def _(v):
            t = 0

            def inc(ins):
                nonlocal t
                t += 1
                return ins.then_inc(s_dve, 1)

            v.memset(ones[:], 1.0).then_inc(s_zp, 1)
            v.memset(zeros_ap(0), 0).then_inc(s_zp, 1)
            # index casts in 2-row granules, ordered by load-sem arrival:
            # pa/pc first (ACT+SP first loads), then pb/pd, tgt c/d (Pool
            # loads), then tgt a/b
            v.wait_ge(s_jf, 1)
            for k in ("pa", "pc", "pb", "pd", "tc", "ta", "tb", "td"):
                side = 0 if k[0] == "p" else 1
                r0 = {"a": 0, "b": 2, "c": 4, "d": 6}[k[1]]
                c0 = side * 1024 + r0 * 128
                v.wait_ge(s_ld[k], 16)
                v.tensor_copy(idx16[:16, c0:c0 + 256],
                              tok32[:, c0:c0 + 256]).then_inc(s_cast[k], 1)

            for r in DVE_SS_ROWS:
                v.wait_ge(s_sc[SC_J[(r, 0)]], 16)
                inc(v.tensor_scalar(out=seg2(junkS, r),
                                    in0=seg2(grid, _grid_blk(r, 0)),
                                    scalar1=0.5, scalar2=None,
                                    op0=mybir.AluOpType.is_gt,
                                    op1=mybir.AluOpType.add,
                                    accum_out=res[:, 8 + r:9 + r]))
            for r in DVE_OV_ROWS:
                v.wait_ge(s_sc[SC_J[(r, 0)]], 16)
                v.wait_ge(s_sc[SC_J[(r, 1)]], 16)
                inc(v.tensor_tensor(out=seg2(junkV, r),
                                    in0=seg2(grid, _grid_blk(r, 0)),
                                    in1=seg2(grid, _grid_blk(r, 1)),
                                    op=mybir.AluOpType.logical_and))
                v.wait_ge(s_dve, t)
                inc(v.tensor_scalar(out=seg2(junkO, r),
                                    in0=seg2(junkV, r),
                                    scalar1=1.0, scalar2=None,
                                    op0=mybir.AluOpType.mult,
                                    op1=mybir.AluOpType.add,
                                    accum_out=res[:, r:r + 1]))
            ticks["final"] = t

        # ---------------- Pool ----------------
        @block.gpsimd
        def _(g):
            g.memset(idx16[:].bitcast(I32), 0).then_inc(s_jf, 1)
            ld2(g, tgt, 1024, 4, s_ld["tc"])
            ld2(g, tgt, 1024, 6, s_ld["td"])
            g.memset(zeros_ap(1), 0).then_inc(s_zt, 1)
            waited = set()
            for j, (r, s) in enumerate(SC_SIDES):
                if j == 0:
                    g.wait_ge(s_zp, 2)
                if s == 1 and "zt" not in waited:
                    waited.add("zt")
                    g.wait_ge(s_zt, 1)
                ck = cast_key(r, s)
                if ck not in waited:
                    waited.add(ck)
                    g.wait_ge(s_cast[ck], 1)
                blk = _grid_blk(r, s)
                ic = (s * 8 + r) * 128
                g.dma_scatter_add(
                    out_ap=grid[:, blk * 256:blk * 256 + 125],
                    out_ap_other=grid[:, blk * 256 + 128:blk * 256 + 253],
                    in_ap=ones[:].rearrange("p (s e) -> p s e", e=1),
                    idxs_ap=idx16[:, ic:ic + 128],
                    num_idxs=L,
                    num_idxs_reg=L,
                    elem_size=1,
                    sbuf_tokens_per_rank=128,
                    parity_reg=0,
                ).then_inc(s_sc[j], 16)
            ticks["povl"] = 0

        # ---------------- ACT ----------------
        @block.scalar
        def _(sc):
            ld2(sc, pred, 0, 0, s_ld["pa"])
            ld2(sc, pred, 0, 2, s_ld["pb"])
            ld2(sc, tgt, 1024, 0, s_ld["ta"])

        # ---------------- SP ----------------
        @block.sync
        def _(sy):
            ld2(sy, pred, 0, 4, s_ld["pc"])
            ld2(sy, pred, 0, 6, s_ld["pd"])
            ld2(sy, tgt, 1024, 2, s_ld["tb"])
            sy.wait_ge(s_dve, ticks["final"])
            sy.wait_ge(s_povl, ticks["povl"])
            sy.dma_start(out=out[:], in_=res[:]).then_inc(s_out, 16)
            sy.wait_ge(s_out, 16)

    nc.compile()
    return nc


def run(pred_tokens, tgt_tokens, trace=False):
    """Returns (loss, exec_time_ns_or_None)."""
    from concourse.bass_utils import run_bass_kernel_spmd

    if "nc" not in _CACHE:
        _CACHE["nc"] = _build_kernel()
    nc = _CACHE["nc"]

    pred_tokens = np.ascontiguousarray(np.asarray(pred_tokens, dtype=np.int32))
    tgt_tokens = np.ascontiguousarray(np.asarray(tgt_tokens, dtype=np.int32))
    assert pred_tokens.shape == (B, L) and tgt_tokens.shape == (B, L)

    in_maps = [
        {
            "pred": pred_tokens[c * ROWS:(c + 1) * ROWS],
            "tgt": tgt_tokens[c * ROWS:(c + 1) * ROWS],
        }
        for c in range(N_CORES)
    ]
    try:
        kres = run_bass_kernel_spmd(nc, in_maps, list(range(N_CORES)),
                                    trace=trace)
    except ModuleNotFoundError:
        kres = run_bass_kernel_spmd(nc, in_maps, list(range(N_CORES)))

    ov = np.empty(B, dtype=np.float64)
    ss = np.empty(B, dtype=np.float64)
    for c, r in enumerate(kres.results):
        o = r["out"]  # [128, 16] f32: cols 0..8 overlap, 8..16 setsize
        ov[c * ROWS:(c + 1) * ROWS] = o[:, :ROWS].sum(axis=0, dtype=np.float64)
        ss[c * ROWS:(c + 1) * ROWS] = o[:, ROWS:].sum(axis=0, dtype=np.float64)

    precision = np.where(ss > 0, ov / np.maximum(ss, 1.0), 0.0)
    loss = np.float32(1.0) - np.float32(precision.mean())
    return loss, kres.exec_time_ns


def kernel(pred_tokens, target_tokens):
    loss, _ = run(pred_tokens, target_tokens)
    return loss


if __name__ == "__main__":
    rng = np.random.default_rng(0)
    p = rng.integers(0, V, (B, L), dtype=np.int32)
    t = rng.integers(0, V, (B, L), dtype=np.int32)
    print(kernel(p, t))
